# revision 19
# baseline (speedup 1.0000x reference)
"""Trainium2 Bass kernel for nn_Calib_48180943127204 (calibration loss_fn).

Data-parallel over B*H across 8 cores (each core: one b, 128 H-rows).
Per-core layout: partitions = (6 row-pair groups x 19 classes) = 114, free = 2048.
  wp = w_c e^{x_c} / S,  S = sum_c w_c e^{x_c}, computed as exp(x - lnS + ln w_c).
Host ships negx = -prediction so ACT's free scale=-1 gives both exps without an
extra negate pass; lnS broadcast lands via a partition-broadcast SBUF->SBUF DMA
with accum_op=add onto the negx tile (t = lnS - x), so the only PE work is the
w-folded class-sum matmul. Counts fuse into one DVE tensor_scalar(is_gt) with
accum_out; squared error = ACT Square(q*vb) with accum_out (vb^2 = vb).
Final tiny cross-core reductions (19 counts + scalar) happen on host.
"""
import sys

if "/opt/trn_rl_repo" not in sys.path:
    sys.path.insert(0, "/opt/trn_rl_repo")

import numpy as np

NUM_CLASSES = 19
THRESHOLD = 0.9
IGNORE = 255
PRIOR = np.array([32.07, 5.71, 20.7, 0.564, 0.761, 1.054, 0.1696, 0.5014, 13.4993,
                  0.8981, 3.6445, 1.1458, 0.1393, 6.0, 0.2949, 0.1954, 0.2341,
                  0.0818, 0.3917], dtype=np.float32)

B, C, H, W = 2, 19, 512, 1024
NCORES = 8
ROWS = 128           # H-rows per core
F = 2048             # free elems per tile (2 rows x 1024)
NTILES = 11          # 10 full [114,F] + 1 partial [76,F]

_compiled = None


def _build_program():
    import concourse.bass as bass
    import concourse.tile as tile
    from concourse import bacc, mybir

    f32 = mybir.dt.float32
    u8 = mybir.dt.uint8

    nc = bacc.Bacc("TRN2", target_bir_lowering=False, debug=False,
                   num_devices=NCORES)

    x_dram = nc.dram_tensor("negx", [C, ROWS, W], f32, kind="ExternalInput")
    lab_dram = nc.dram_tensor("lab", [C, ROWS, W], u8, kind="ExternalInput")
    w_dram = nc.dram_tensor("wvec", [C, 1], f32, kind="ExternalInput")
    iota_dram = nc.dram_tensor("iota", [114, 1], f32, kind="ExternalInput")
    mask6_dram = nc.dram_tensor("mask6", [114, 6], f32, kind="ExternalInput")

    wp_dram = nc.dram_tensor("wp", [C, ROWS, W], f32, kind="ExternalOutput")
    cnt_dram = nc.dram_tensor("cnt", [114, NTILES], f32, kind="ExternalOutput")
    se_dram = nc.dram_tensor("se", [114, NTILES], f32, kind="ExternalOutput")

    with tile.TileContext(nc) as tc:
        from contextlib import ExitStack
        with ExitStack() as ctx:
            const_pool = ctx.enter_context(tc.tile_pool(name="const", bufs=1))
            xt_pool = ctx.enter_context(tc.tile_pool(name="xt", bufs=3))
            lnS_pool = ctx.enter_context(tc.tile_pool(name="lnS", bufs=2))
            e_pool = ctx.enter_context(tc.tile_pool(name="e", bufs=2))
            lab_pool = ctx.enter_context(tc.tile_pool(name="lab", bufs=2))
            wp_pool = ctx.enter_context(tc.tile_pool(name="wp", bufs=3))
            gt_pool = ctx.enter_context(tc.tile_pool(name="gt", bufs=2))
            oh_pool = ctx.enter_context(tc.tile_pool(name="oh", bufs=2))
            vb_pool = ctx.enter_context(tc.tile_pool(name="vb", bufs=2))
            q_pool = ctx.enter_context(tc.tile_pool(name="q", bufs=2))
            qm_pool = ctx.enter_context(tc.tile_pool(name="qm", bufs=2))
            sq_pool = ctx.enter_context(tc.tile_pool(name="sq", bufs=2))
            psA_pool = ctx.enter_context(
                tc.tile_pool(name="psA", bufs=2, space=bass.MemorySpace.PSUM))

            iota = const_pool.tile([114, 1], f32)
            nc.sync.dma_start(iota[:], iota_dram.ap())
            mask6 = const_pool.tile([114, 6], f32)
            nc.sync.dma_start(mask6[:], mask6_dram.ap())
            wcol = const_pool.tile([114, 1], f32)
            for g in range(6):
                nc.sync.dma_start(wcol[19 * g:19 * (g + 1), :], w_dram.ap())
            lnw = const_pool.tile([114, 1], f32)
            nc.scalar.activation(lnw[:], wcol[:], mybir.ActivationFunctionType.Ln)

            cnt_cols = const_pool.tile([114, NTILES], f32)
            nc.vector.memset(cnt_cols[:], 0.0)
            se_cols = const_pool.tile([114, NTILES], f32)
            nc.vector.memset(se_cols[:], 0.0)

            for t in range(NTILES):
                r0 = 12 * t
                rows = 12 if t < NTILES - 1 else 8
                G = rows // 2
                P = G * 19

                def gc_view(dram):
                    return dram.ap()[:, r0:r0 + rows, :].rearrange(
                        "c (g r) w -> c g (r w)", g=G, r=2).transpose((1, 0, 2))

                xt = xt_pool.tile([P, F], f32, tag="xt")
                nc.sync.dma_start(xt[:], gc_view(x_dram))
                labf = lab_pool.tile([P, F], f32, tag="lab")
                nc.gpsimd.dma_start(labf[:], gc_view(lab_dram))  # u8->f32 cast

                # e' = w_c * exp(x) = Exp(-1 * negx + ln w_c)
                e = e_pool.tile([P, F], f32, tag="e")
                nc.scalar.activation(e[:], xt[:],
                                     mybir.ActivationFunctionType.Exp,
                                     bias=lnw[:P, :], scale=-1.0)

                psA = psA_pool.tile([G, F], f32, tag="psA")
                for k in range(F // 512):
                    nc.tensor.matmul(psA[:, 512 * k:512 * (k + 1)],
                                     mask6[:P, :G],
                                     e[:, 512 * k:512 * (k + 1)],
                                     start=True, stop=True)
                lnS = lnS_pool.tile([G, F], f32, tag="lnS")
                nc.scalar.activation(lnS[:], psA[:],
                                     mybir.ActivationFunctionType.Ln)

                # xt becomes t = -x + lnS via partition-broadcast accum DMA
                nc.gpsimd.dma_start(
                    xt[:],
                    lnS[:].unsqueeze(1).broadcast_to((G, 19, F)),
                    accum_op=mybir.AluOpType.add)

                # wp = Exp(-1 * t + ln w_c) = exp(x - lnS + ln w_c)
                wp = wp_pool.tile([P, F], f32, tag="wp")
                nc.scalar.activation(wp[:], xt[:],
                                     mybir.ActivationFunctionType.Exp,
                                     bias=lnw[:P, :], scale=-1.0)
                nc.sync.dma_start(gc_view(wp_dram), wp[:])

                # counts: one fused DVE op
                gt = gt_pool.tile([P, F], f32, tag="gt")
                nc.vector.tensor_scalar(gt[:], wp[:], THRESHOLD, None,
                                        op0=mybir.AluOpType.is_gt,
                                        op1=mybir.AluOpType.add,
                                        accum_out=cnt_cols[:P, t:t + 1])

                oh = oh_pool.tile([P, F], f32, tag="oh")
                nc.vector.tensor_scalar(oh[:], labf[:], iota[:P, :], None,
                                        op0=mybir.AluOpType.is_equal)
                vb = vb_pool.tile([P, F], f32, tag="vb")
                nc.vector.tensor_scalar(vb[:], labf[:], float(IGNORE), None,
                                        op0=mybir.AluOpType.not_equal)
                q = q_pool.tile([P, F], f32, tag="q")
                nc.vector.tensor_tensor(q[:], wp[:], oh[:],
                                        op=mybir.AluOpType.subtract)
                qm = qm_pool.tile([P, F], f32, tag="qm")
                nc.vector.tensor_tensor(qm[:], q[:], vb[:],
                                        op=mybir.AluOpType.mult)
                # Square(qm) = q^2 * vb^2 = q^2 * vb (vb is 0/1), fused reduce
                sq = sq_pool.tile([P, F], f32, tag="sq")
                nc.scalar.activation(sq[:], qm[:],
                                     mybir.ActivationFunctionType.Square,
                                     accum_out=se_cols[:P, t:t + 1])

            nc.sync.dma_start(cnt_dram.ap(), cnt_cols[:])
            nc.sync.dma_start(se_dram.ap(), se_cols[:])

    nc.compile()
    return nc


def _host_constants():
    p = np.arange(114)
    iota = (p % 19).astype(np.float32).reshape(114, 1)
    mask6 = np.zeros((114, 6), np.float32)
    mask6[p, p // 19] = 1.0
    return iota, mask6


def build_in_maps(prediction, weight, label):
    prediction = np.asarray(prediction, dtype=np.float32)
    weight = np.asarray(weight, dtype=np.float32)
    label_np = np.asarray(label)
    iota, mask6 = _host_constants()
    in_maps = []
    for k in range(NCORES):
        b, j = k // 4, k % 4
        negx = np.ascontiguousarray(-prediction[b, :, j * ROWS:(j + 1) * ROWS, :])
        lab = label_np[b, j * ROWS:(j + 1) * ROWS, :].astype(np.uint8)
        lab_rep = np.ascontiguousarray(np.broadcast_to(lab[None], (C, ROWS, W)))
        in_maps.append({
            "negx": negx, "lab": lab_rep, "wvec": weight.reshape(C, 1),
            "iota": iota, "mask6": mask6,
        })
    return in_maps


_last_in_maps = None


def kernel(prediction, weight, label):
    global _compiled, _last_in_maps
    from concourse.bass_utils import run_bass_kernel_spmd

    if _compiled is None:
        _compiled = _build_program()
    nc = _compiled

    in_maps = build_in_maps(prediction, weight, label)
    _last_in_maps = in_maps

    results = run_bass_kernel_spmd(nc, in_maps, list(range(NCORES))).results

    wp_full = np.zeros((B, C, H, W), np.float32)
    cnt_sum = np.zeros((NCORES, 114), np.float64)
    se_sum = 0.0
    for k in range(NCORES):
        b, j = k // 4, k % 4
        r = results[k]
        wp_full[b, :, j * ROWS:(j + 1) * ROWS, :] = r["wp"]
        cnt_sum[k] = r["cnt"].astype(np.float64).sum(axis=1)
        se_sum += r["se"].astype(np.float64).sum()

    size_all = np.zeros(19, np.float64)
    for c in range(19):
        size_all[c] = cnt_sum[:, c::19].sum()

    prior_ratio = (PRIOR / PRIOR.sum()).astype(np.float64)
    ratio_all = size_all / size_all.sum()
    ratio_loss = np.sum((ratio_all - prior_ratio) ** 2)
    mse = se_sum / float(B * H * W)
    all_loss = np.float32(ratio_loss + 0.05 * mse)
    return all_loss, wp_full


# revision 20
# speedup vs baseline: 2.1408x; 2.1408x over previous
"""Trainium2 Bass kernel for nn_Calib_48180943127204 (calibration loss_fn).

Data-parallel over B*H across 8 cores (each core: one b, 128 H-rows).
Per-core layout: partitions = (6 row-pair groups x 19 classes) = 114, free = 2048.
  wp = w_c e^{x_c} / S,  S = sum_c w_c e^{x_c}, computed as exp(x - lnS + ln w_c).
Host ships negx = -prediction so ACT's free scale=-1 gives both exps without an
extra negate pass; lnS broadcast lands via a partition-broadcast SBUF->SBUF DMA
with accum_op=add onto the negx tile (t = lnS - x), so the only PE work is the
w-folded class-sum matmul. Counts fuse into one DVE tensor_scalar(is_gt) with
accum_out; squared error = ACT Square(q*vb) with accum_out (vb^2 = vb).
Final tiny cross-core reductions (19 counts + scalar) happen on host.
"""
import sys

if "/opt/trn_rl_repo" not in sys.path:
    sys.path.insert(0, "/opt/trn_rl_repo")

import numpy as np

NUM_CLASSES = 19
THRESHOLD = 0.9
IGNORE = 255
PRIOR = np.array([32.07, 5.71, 20.7, 0.564, 0.761, 1.054, 0.1696, 0.5014, 13.4993,
                  0.8981, 3.6445, 1.1458, 0.1393, 6.0, 0.2949, 0.1954, 0.2341,
                  0.0818, 0.3917], dtype=np.float32)

B, C, H, W = 2, 19, 512, 1024
NCORES = 8
ROWS = 128           # H-rows per core
F = 2048             # free elems per tile (2 rows x 1024)
NTILES = 11          # 10 full [114,F] + 1 partial [76,F]

_compiled = None


def _build_program():
    import concourse.bass as bass
    import concourse.tile as tile
    from concourse import bacc, mybir

    f32 = mybir.dt.float32
    u8 = mybir.dt.uint8

    nc = bacc.Bacc("TRN2", target_bir_lowering=False, debug=False,
                   num_devices=NCORES)

    x_dram = nc.dram_tensor("negx", [C, ROWS, W], f32, kind="ExternalInput")
    lab_dram = nc.dram_tensor("lab", [C, ROWS, W], u8, kind="ExternalInput")
    w_dram = nc.dram_tensor("wvec", [C, 1], f32, kind="ExternalInput")
    iota_dram = nc.dram_tensor("iota", [114, 1], f32, kind="ExternalInput")
    bmask_dram = nc.dram_tensor("bmask", [114, 114], f32, kind="ExternalInput")

    wp_dram = nc.dram_tensor("wp", [C, ROWS, W], f32, kind="ExternalOutput")
    cnt_dram = nc.dram_tensor("cnt", [114, NTILES], f32, kind="ExternalOutput")
    se_dram = nc.dram_tensor("se", [114, NTILES], f32, kind="ExternalOutput")

    with tile.TileContext(nc) as tc:
        from contextlib import ExitStack
        with ExitStack() as ctx:
            const_pool = ctx.enter_context(tc.tile_pool(name="const", bufs=1))
            xt_pool = ctx.enter_context(tc.tile_pool(name="xt", bufs=3))
            lnS_pool = ctx.enter_context(tc.tile_pool(name="lnS", bufs=2))
            e_pool = ctx.enter_context(tc.tile_pool(name="e", bufs=2))
            lab_pool = ctx.enter_context(tc.tile_pool(name="lab", bufs=2))
            wp_pool = ctx.enter_context(tc.tile_pool(name="wp", bufs=3))
            gt_pool = ctx.enter_context(tc.tile_pool(name="gt", bufs=1))
            oh_pool = ctx.enter_context(tc.tile_pool(name="oh", bufs=2))
            vb_pool = ctx.enter_context(tc.tile_pool(name="vb", bufs=2))
            q_pool = ctx.enter_context(tc.tile_pool(name="q", bufs=2))
            qm_pool = ctx.enter_context(tc.tile_pool(name="qm", bufs=2))
            sq_pool = ctx.enter_context(tc.tile_pool(name="sq", bufs=1))
            psA_pool = ctx.enter_context(
                tc.tile_pool(name="psA", bufs=2, space=bass.MemorySpace.PSUM))

            iota = const_pool.tile([114, 1], f32)
            nc.sync.dma_start(iota[:], iota_dram.ap())
            bmask = const_pool.tile([114, 114], f32)
            nc.sync.dma_start(bmask[:], bmask_dram.ap())
            wcol = const_pool.tile([114, 1], f32)
            for g in range(6):
                nc.sync.dma_start(wcol[19 * g:19 * (g + 1), :], w_dram.ap())
            lnw = const_pool.tile([114, 1], f32)
            nc.scalar.activation(lnw[:], wcol[:], mybir.ActivationFunctionType.Ln)

            cnt_cols = const_pool.tile([114, NTILES], f32)
            nc.vector.memset(cnt_cols[:], 0.0)
            se_cols = const_pool.tile([114, NTILES], f32)
            nc.vector.memset(se_cols[:], 0.0)

            for t in range(NTILES):
                r0 = 12 * t
                rows = 12 if t < NTILES - 1 else 8
                G = rows // 2
                P = G * 19

                def gc_view(dram):
                    return dram.ap()[:, r0:r0 + rows, :].rearrange(
                        "c (g r) w -> c g (r w)", g=G, r=2).transpose((1, 0, 2))

                xt = xt_pool.tile([P, F], f32, tag="xt")
                nc.sync.dma_start(xt[:], gc_view(x_dram))
                labf = lab_pool.tile([P, F], f32, tag="lab")
                nc.gpsimd.dma_start(labf[:], gc_view(lab_dram))  # u8->f32 cast

                # e' = w_c * exp(x) = Exp(-1 * negx + ln w_c)
                e = e_pool.tile([P, F], f32, tag="e")
                nc.scalar.activation(e[:], xt[:],
                                     mybir.ActivationFunctionType.Exp,
                                     bias=lnw[:P, :], scale=-1.0)

                # S broadcast to all partitions: psS[m,f] = sum_{p: g(p)=g(m)} e[p,f]
                psS = psA_pool.tile([P, F], f32, tag="psA")
                for k in range(F // 512):
                    nc.tensor.matmul(psS[:, 512 * k:512 * (k + 1)],
                                     bmask[:P, :P],
                                     e[:, 512 * k:512 * (k + 1)],
                                     start=True, stop=True)
                lnSb = lnS_pool.tile([P, F], f32, tag="lnS")
                nc.scalar.activation(lnSb[:], psS[:],
                                     mybir.ActivationFunctionType.Ln)

                # z = lnS - x  (xt holds -x)
                z = e_pool.tile([P, F], f32, tag="e")
                nc.vector.tensor_tensor(z[:], xt[:], lnSb[:],
                                        op=mybir.AluOpType.add)

                # wp = Exp(-1 * z + ln w_c) = exp(x - lnS + ln w_c)
                wp = wp_pool.tile([P, F], f32, tag="wp")
                nc.scalar.activation(wp[:], z[:],
                                     mybir.ActivationFunctionType.Exp,
                                     bias=lnw[:P, :], scale=-1.0)
                nc.sync.dma_start(gc_view(wp_dram), wp[:])

                # counts: one fused DVE op
                gt = gt_pool.tile([P, F], f32, tag="gt")
                nc.vector.tensor_scalar(gt[:], wp[:], THRESHOLD, None,
                                        op0=mybir.AluOpType.is_gt,
                                        op1=mybir.AluOpType.add,
                                        accum_out=cnt_cols[:P, t:t + 1])

                oh = oh_pool.tile([P, F], f32, tag="oh")
                nc.vector.tensor_scalar(oh[:], labf[:], iota[:P, :], None,
                                        op0=mybir.AluOpType.is_equal)
                vb = vb_pool.tile([P, F], f32, tag="vb")
                nc.vector.tensor_scalar(vb[:], labf[:], float(IGNORE), None,
                                        op0=mybir.AluOpType.not_equal)
                q = q_pool.tile([P, F], f32, tag="q")
                nc.vector.tensor_tensor(q[:], wp[:], oh[:],
                                        op=mybir.AluOpType.subtract)
                qm = qm_pool.tile([P, F], f32, tag="qm")
                nc.vector.tensor_tensor(qm[:], q[:], vb[:],
                                        op=mybir.AluOpType.mult)
                # Square(qm) = q^2 * vb^2 = q^2 * vb (vb is 0/1), fused reduce
                sq = sq_pool.tile([P, F], f32, tag="sq")
                nc.scalar.activation(sq[:], qm[:],
                                     mybir.ActivationFunctionType.Square,
                                     accum_out=se_cols[:P, t:t + 1])

            nc.sync.dma_start(cnt_dram.ap(), cnt_cols[:])
            nc.sync.dma_start(se_dram.ap(), se_cols[:])

    nc.compile()
    return nc


def _host_constants():
    p = np.arange(114)
    iota = (p % 19).astype(np.float32).reshape(114, 1)
    bmask = (p[:, None] // 19 == p[None, :] // 19).astype(np.float32)
    return iota, bmask


def build_in_maps(prediction, weight, label):
    prediction = np.asarray(prediction, dtype=np.float32)
    weight = np.asarray(weight, dtype=np.float32)
    label_np = np.asarray(label)
    iota, bmask = _host_constants()
    in_maps = []
    for k in range(NCORES):
        b, j = k // 4, k % 4
        negx = np.ascontiguousarray(-prediction[b, :, j * ROWS:(j + 1) * ROWS, :])
        lab = label_np[b, j * ROWS:(j + 1) * ROWS, :].astype(np.uint8)
        lab_rep = np.ascontiguousarray(np.broadcast_to(lab[None], (C, ROWS, W)))
        in_maps.append({
            "negx": negx, "lab": lab_rep, "wvec": weight.reshape(C, 1),
            "iota": iota, "bmask": bmask,
        })
    return in_maps


_last_in_maps = None


def kernel(prediction, weight, label):
    global _compiled, _last_in_maps
    from concourse.bass_utils import run_bass_kernel_spmd

    if _compiled is None:
        _compiled = _build_program()
    nc = _compiled

    in_maps = build_in_maps(prediction, weight, label)
    _last_in_maps = in_maps

    results = run_bass_kernel_spmd(nc, in_maps, list(range(NCORES))).results

    wp_full = np.zeros((B, C, H, W), np.float32)
    cnt_sum = np.zeros((NCORES, 114), np.float64)
    se_sum = 0.0
    for k in range(NCORES):
        b, j = k // 4, k % 4
        r = results[k]
        wp_full[b, :, j * ROWS:(j + 1) * ROWS, :] = r["wp"]
        cnt_sum[k] = r["cnt"].astype(np.float64).sum(axis=1)
        se_sum += r["se"].astype(np.float64).sum()

    size_all = np.zeros(19, np.float64)
    for c in range(19):
        size_all[c] = cnt_sum[:, c::19].sum()

    prior_ratio = (PRIOR / PRIOR.sum()).astype(np.float64)
    ratio_all = size_all / size_all.sum()
    ratio_loss = np.sum((ratio_all - prior_ratio) ** 2)
    mse = se_sum / float(B * H * W)
    all_loss = np.float32(ratio_loss + 0.05 * mse)
    return all_loss, wp_full
